# revision 16
# baseline (speedup 1.0000x reference)
"""LeNet-style ClientNet (dense_cnn) on 8 Trainium2 NeuronCores.

Strategy (data-parallel, batch sharded 8x1024). The wall clock is dominated
by the axon tunnel (~80 MB/s, ~100 ms per extra input array) and per-call
client-side lowering, so the design minimizes host->device bytes and array
count, not FLOPs:
  host: ps-weighted average of the 16 client stacks (tiny einsum), weights
        pre-shaped into banded lhsT layouts for the PE; ONE f32 blob per
        core: x as fp8-e4m3 (~0.8 MB), conv/fc1 weights bf16, fc2 f32
        (~1.8 MB total vs 24 MB host-staged f32 originally). A persistent
        jax compilation cache skips the per-call walrus recompile.
  core: im2col for conv1 via strided fp8 DMAs from the blob (30 per
        32-sample chunk) upconverted to bf16 on DVE, conv1 as 9 K=41xN=512
        bf16 matmuls per chunk (banded rows + ones row for bias) with relu
        fused into the psum evict and one pool-x DVE op, conv2 as 5
        dx-accumulated K=121 bf16 matmuls per 16 samples, fc1 as 16
        accumulated K=51 bf16 matmuls (one per spatial tap), fc2 K=126 x4
        in f32r, f16 output. psum always fp32. A strict all-engine barrier
        before each fc group: without it fc1 raced Y2 writes on HW (passes
        CoreSim; diverges on silicon).
"""

import contextlib
import sys

import numpy as np
import ml_dtypes

sys.path.insert(0, "/opt/trn_rl_repo")

import jax  # noqa: E402

# Persistent executable cache: the bass_exec HLO (which embeds the BIR) is
# byte-stable across calls, so cached executables skip the per-call walrus
# recompile (~0.3 s/call).
try:
    jax.config.update("jax_compilation_cache_dir", "/root/.jax_comp_cache")
    jax.config.update("jax_persistent_cache_min_compile_time_secs", 0.0)
    jax.config.update("jax_persistent_cache_min_entry_size_bytes", 0)
except Exception:
    pass

import concourse.bass as bass  # noqa: E402
import concourse.bacc as bacc  # noqa: E402
import concourse.mybir as mybir  # noqa: E402
from concourse.tile import TileContext  # noqa: E402

F32R = mybir.dt.float32r
F32 = mybir.dt.float32
F16 = mybir.dt.float16
BF16 = mybir.dt.bfloat16
FP8 = mybir.dt.float8e4
MAX = mybir.AluOpType.max
ADD = mybir.AluOpType.add
BFNP = ml_dtypes.bfloat16
F8NP = ml_dtypes.float8_e4m3

NCORES = 8
BC = 1024            # samples per core
CH = 32              # samples per chunk
NCH = BC // CH       # 32 chunks
QC = 8               # chunks per fc group (256 samples)
NQ = NCH // QC       # 4 fc groups

# ---- single-blob layout (units: f32 elements; bf16 sections hold 2/elem,
# fp8 sections hold 4/elem) ----
OFF_X = 0                       # x fp8 [1024,784]
XF = BC * 784 // 4              # 200704
OFF_ON8 = OFF_X + XF            # fp8 ones [4608]
ON8F = 4608 // 4                # 1152
OFF_L1 = OFF_ON8 + ON8F         # conv1 lhsT bf16 [41,104]
L1F = 41 * 104 // 2             # 2132
OFF_L2 = OFF_L1 + L1F           # conv2 lhsT bf16 [121,570]
L2F = 121 * 570 // 2            # 34485
OFF_LF1 = OFF_L2 + L2F          # fc1 lhsT bf16 [51,8000]
LF1F = 51 * 8000 // 2           # 204000
OFF_LF2 = OFF_LF1 + LF1F        # fc2 lhsT f32 [126,40]
LF2F = 126 * 40                 # 5040
OFF_ON32 = OFF_LF2 + LF2F       # f32 ones [4096]
ON32F = 4096
OFF_ON16 = OFF_ON32 + ON32F     # bf16 ones [8192]
ON16F = 8192 // 2
NTOT = OFF_ON16 + ON16F         # 655257


def _ap(t, off, dims):
    return bass.AP(tensor=t.tensor, offset=t.offset + off, ap=[list(d) for d in dims])


def _pitch(t):
    return t.ap[0][0]


def build_host_weights(ps, conv1_w, conv1_b, conv2_w, conv2_b,
                       fc1_w, fc1_b, fc2_w, fc2_b):
    ps = np.asarray(ps, np.float64)
    W1 = np.einsum("n,noihw->oihw", ps, np.asarray(conv1_w, np.float64))[:, 0]  # [20,5,5]
    b1 = ps @ np.asarray(conv1_b, np.float64)                                   # [20]
    W2 = np.einsum("n,noihw->oihw", ps, np.asarray(conv2_w, np.float64))        # [50,20,5,5]
    b2 = ps @ np.asarray(conv2_b, np.float64)                                   # [50]
    Wf1 = np.einsum("n,nof->of", ps, np.asarray(fc1_w, np.float64))             # [500,800]
    bf1 = ps @ np.asarray(fc1_b, np.float64)                                    # [500]
    Wf2 = np.einsum("n,nof->of", ps, np.asarray(fc2_w, np.float64))             # [10,500]
    bf2 = ps @ np.asarray(fc2_b, np.float64)                                    # [10]

    # conv1 lhsT [41, 104]: k = dx*8 + rr (rows 0..39), row 40 = bias ones-row.
    # m = e*64 + u*20 + o ; out row y = 4G + 2u + e ; input row 4G + rr,
    # dy = rr - (2u + e) in 0..4.
    L1 = np.zeros((41, 104), np.float32)
    for dx in range(5):
        for rr in range(8):
            for e in range(2):
                for u in range(2):
                    for o in range(20):
                        dy = rr - (2 * u + e)
                        if 0 <= dy <= 4:
                            L1[dx * 8 + rr, e * 64 + u * 20 + o] = W1[o, dy, dx]
    for e in range(2):
        for u in range(2):
            for o in range(20):
                L1[40, e * 64 + u * 20 + o] = b1[o]

    # conv2 lhsT [121, 570]: k = rr*20 + c, m(dx) = dx*114 + e*64 + o.
    # out row y' = 2gg + e ; pooled input row 2gg + rr ; dy = rr - e.
    L2 = np.zeros((121, 570), np.float32)
    for dx in range(5):
        for c in range(20):
            for rr in range(6):
                for e in range(2):
                    dy = rr - e
                    if 0 <= dy <= 4:
                        L2[rr * 20 + c, dx * 114 + e * 64:dx * 114 + e * 64 + 50] = \
                            W2[:, c, dy, dx]
    for e in range(2):
        L2[120, e * 64:e * 64 + 50] = b2

    # fc1 lhsT [51, 16*500]: tap f = gg*4 + xp; torch feature id = o*16 + f.
    LF1 = np.zeros((51, 8000), np.float32)
    for gg in range(4):
        for xp in range(4):
            f = gg * 4 + xp
            for o in range(50):
                LF1[o, f * 500:(f + 1) * 500] = Wf1[:, o * 16 + f]
    LF1[50, 0:500] = bf1

    # fc2 lhsT [126, 40]
    LF2 = np.zeros((126, 40), np.float32)
    for c in range(4):
        LF2[0:125, c * 10:(c + 1) * 10] = Wf2[:, c * 125:(c + 1) * 125].T
    LF2[125, 0:10] = bf2

    # ---- pack weight sections of the blob (f32 view) ----
    wsec = np.zeros(NTOT - OFF_ON8, np.float32)

    def put16(off_f32, arr):
        v = np.ascontiguousarray(arr.astype(BFNP)).reshape(-1).view(np.float32)
        wsec[off_f32 - OFF_ON8:off_f32 - OFF_ON8 + v.size] = v

    v8 = np.ones(4608, F8NP).reshape(-1).view(np.float32)
    wsec[0:ON8F] = v8
    put16(OFF_L1, L1)
    put16(OFF_L2, L2)
    put16(OFF_LF1, LF1)
    wsec[OFF_LF2 - OFF_ON8:OFF_LF2 - OFF_ON8 + LF2F] = LF2.reshape(-1)
    wsec[OFF_ON32 - OFF_ON8:OFF_ON32 - OFF_ON8 + ON32F] = 1.0
    put16(OFF_ON16, np.ones(8192, np.float32))
    return wsec


def build_in_maps(x, ps, conv1_w, conv1_b, conv2_w, conv2_b,
                  fc1_w, fc1_b, fc2_w, fc2_b):
    wsec = build_host_weights(ps, conv1_w, conv1_b, conv2_w, conv2_b,
                              fc1_w, fc1_b, fc2_w, fc2_b)
    x8 = np.asarray(x, np.float32).reshape(NCORES, BC * 784).astype(F8NP)
    in_maps = []
    for c in range(NCORES):
        blob = np.empty(NTOT, np.float32)
        blob[OFF_X:OFF_X + XF] = x8[c].view(np.float32)
        blob[OFF_ON8:] = wsec
        in_maps.append({"blob": blob})
    return in_maps


def build_nc():
    nc = bacc.Bacc()
    blob_d = nc.dram_tensor("blob", [NTOT], F32R, kind="ExternalInput")
    out_d = nc.dram_tensor("out", [BC, 10], F16, kind="ExternalOutput")
    b32 = blob_d[:]
    b16 = blob_d.bitcast(BF16)[:]
    b8 = blob_d.bitcast(FP8)[:]

    ctx = contextlib.ExitStack()
    with ctx:
        with TileContext(nc) as tc:
            with contextlib.ExitStack() as pctx:
                cpool = pctx.enter_context(tc.tile_pool(name="const", bufs=1))
                r8p = pctx.enter_context(tc.tile_pool(name="r8", bufs=2))
                r1p = pctx.enter_context(tc.tile_pool(name="r1", bufs=2))
                p1p = pctx.enter_context(tc.tile_pool(name="p1", bufs=2))
                y1p = pctx.enter_context(tc.tile_pool(name="y1", bufs=2))
                c2rp = pctx.enter_context(tc.tile_pool(name="c2r", bufs=2))
                p2p = pctx.enter_context(tc.tile_pool(name="p2", bufs=2))
                t2p = pctx.enter_context(tc.tile_pool(name="t2", bufs=2))
                y2p = pctx.enter_context(tc.tile_pool(name="y2", bufs=2))
                y3p = pctx.enter_context(tc.tile_pool(name="y3", bufs=2))
                osbp = pctx.enter_context(tc.tile_pool(name="osb", bufs=2))
                p1ep = pctx.enter_context(tc.tile_pool(name="p1e", bufs=2))
                p1bp = pctx.enter_context(tc.tile_pool(name="p1b", bufs=2))
                p2bp = pctx.enter_context(tc.tile_pool(name="p2b", bufs=2))
                e2p = pctx.enter_context(tc.tile_pool(name="e2", bufs=2))
                ps1p = pctx.enter_context(tc.tile_pool(name="ps1", bufs=2, space="PSUM"))
                ps2p = pctx.enter_context(tc.tile_pool(name="ps2", bufs=2, space="PSUM"))
                ps3p = pctx.enter_context(tc.tile_pool(name="ps3", bufs=2, space="PSUM"))
                ps4p = pctx.enter_context(tc.tile_pool(name="ps4", bufs=2, space="PSUM"))
                # --- constants ---
                L1 = cpool.tile([41, 104], BF16)
                nc.sync.dma_start(
                    out=L1[:, :], in_=_ap(b16, 2 * OFF_L1, [[104, 41], [1, 104]]))
                L2 = cpool.tile([121, 570], BF16)
                nc.sync.dma_start(
                    out=L2[:, :], in_=_ap(b16, 2 * OFF_L2, [[570, 121], [1, 570]]))
                LF1 = cpool.tile([51, 8000], BF16)
                nc.sync.dma_start(
                    out=LF1[:, :], in_=_ap(b16, 2 * OFF_LF1, [[8000, 51], [1, 8000]]))
                LF2 = cpool.tile([126, 40], F32R)
                nc.sync.dma_start(
                    out=LF2[:, :], in_=_ap(b32, OFF_LF2, [[40, 126], [1, 40]]))

                y2_cur = None
                c2r_tiles = []
                for j in range(2):
                    t_ = c2rp.tile([121, CH * 48], BF16)
                    nc.sync.dma_start(
                        out=_ap(t_[:, :], 120 * _pitch(t_[:, :]),
                                [[_pitch(t_[:, :]), 1], [1, CH * 48]]),
                        in_=_ap(b16, 2 * OFF_ON16, [[0, 1], [1, CH * 48]]),
                    )
                    c2r_tiles.append(t_)
                for i in range(NCH):
                    q = i // QC
                    # ---- conv1 rhs: on-device im2col in fp8 (30 DMAs + ones
                    # row), then one gpsimd upconvert to bf16 ----
                    R8 = r8p.tile([41, CH * 144], FP8)
                    p8 = _pitch(R8[:, :])
                    nc.sync.dma_start(
                        out=_ap(R8[:, :], 40 * p8, [[p8, 1], [1, CH * 144]]),
                        in_=_ap(b8, 4 * OFF_ON8, [[0, 1], [1, CH * 144]]),
                    )
                    for dx in range(5):
                        for g in range(6):
                            eng = nc.sync if (dx * 6 + g) % 2 == 0 else nc.scalar
                            eng.dma_start(
                                out=_ap(R8[:, :], dx * 8 * p8 + g * 24,
                                        [[p8, 8], [144, CH], [1, 24]]),
                                in_=_ap(b8, 4 * OFF_X + i * CH * 784 + g * 112 + dx,
                                        [[28, 8], [784, CH], [1, 24]]),
                            )
                    R1 = r1p.tile([41, CH * 144], BF16)
                    pr = _pitch(R1[:, :])
                    nc.vector.tensor_copy(out=R1[:, :], in_=R8[:, :])
                    # ---- conv1: 9 N=512 matmuls, relu fused into the psum evict,
                    # one pool-x DVE op for the whole chunk ----
                    P1E = p1ep.tile([104, CH * 144], BF16)
                    ppe = _pitch(P1E[:, :])
                    for bs in range(CH * 144 // 512):
                        ps1 = ps1p.tile([104, 512], F32)
                        nc.tensor.matmul(
                            ps1[:, :], L1[:, :],
                            _ap(R1[:, :], bs * 512, [[pr, 41], [1, 512]]),
                            start=True, stop=True,
                        )
                        nc.scalar.activation(
                            out=P1E[:, bs * 512:(bs + 1) * 512], in_=ps1[:, :],
                            func=mybir.ActivationFunctionType.Relu)
                    P1 = p1p.tile([104, CH * 72], BF16)
                    pp1 = _pitch(P1[:, :])
                    nc.vector.tensor_tensor(
                        out=_ap(P1[:, :], 0,
                                [[pp1, 104], [72, CH], [12, 6], [1, 12]]),
                        in0=_ap(P1E[:, :], 0,
                                [[ppe, 104], [144, CH], [24, 6], [2, 12]]),
                        in1=_ap(P1E[:, :], 1,
                                [[ppe, 104], [144, CH], [24, 6], [2, 12]]),
                        op=MAX,
                    )
                    # ---- conv1 pool-y (inputs already relu'd) ----
                    P1B = p1bp.tile([40, CH * 72], BF16)
                    nc.sync.dma_start(out=P1B[:, :], in_=P1[64:104, :])
                    Y1 = y1p.tile([40, CH * 72], BF16)
                    nc.vector.tensor_tensor(
                        out=Y1[:, :], in0=P1[0:40, :], in1=P1B[:, :], op=MAX)
                    # ---- shuffle Y1 -> C2R (6 DMAs) ----
                    C2R = c2r_tiles[i % 2]
                    pc = _pitch(C2R[:, :])
                    py1 = _pitch(Y1[:, :])
                    for u in range(2):
                        for v in range(3):
                            nc.sync.dma_start(
                                out=_ap(C2R[:, :], (2 * v + u) * 20 * pc,
                                        [[pc, 20], [48, CH], [1, 48]]),
                                in_=_ap(Y1[:, :], u * 20 * py1 + v * 12,
                                        [[py1, 20], [72, CH], [1, 48]]),
                            )
                    # ---- conv2: groups of 16 samples ----
                    P2 = p2p.tile([114, CH * 16], BF16)
                    pp2 = _pitch(P2[:, :])
                    for bg in range(CH // 16):
                        ps2 = ps2p.tile([114, 512], F32)
                        for dx in range(5):
                            nc.tensor.matmul(
                                ps2[:, :],
                                _ap(L2[:, :], dx * 114,
                                    [[_pitch(L2[:, :]), 121], [1, 114]]),
                                _ap(C2R[:, :], bg * 16 * 48 + dx,
                                    [[pc, 121], [48, 16], [12, 4], [1, 8]]),
                                start=(dx == 0), stop=(dx == 4),
                            )
                        E2 = e2p.tile([114, 512], BF16)
                        pe2 = _pitch(E2[:, :])
                        nc.scalar.copy(out=E2[:, :], in_=ps2[:, :])
                        nc.vector.tensor_tensor(
                            out=_ap(P2[:, :], bg * 256,
                                    [[pp2, 114], [16, 16], [4, 4], [1, 4]]),
                            in0=_ap(E2[:, :], 0,
                                    [[pe2, 114], [32, 16], [8, 4], [2, 4]]),
                            in1=_ap(E2[:, :], 1,
                                    [[pe2, 114], [32, 16], [8, 4], [2, 4]]),
                            op=MAX,
                        )
                    # ---- conv2 pool-y + bias/relu into Y2 ----
                    P2B = p2bp.tile([50, CH * 16], BF16)
                    nc.sync.dma_start(out=P2B[:, :], in_=P2[64:114, :])
                    T2 = t2p.tile([50, CH * 16], BF16)
                    nc.vector.tensor_tensor(
                        out=T2[:, :], in0=P2[0:50, :], in1=P2B[:, :], op=MAX)
                    if i % QC == 0:
                        y2_cur = y2p.tile([51, QC * CH * 16], BF16)
                        nc.sync.dma_start(
                            out=_ap(y2_cur[:, :], 50 * _pitch(y2_cur[:, :]),
                                    [[_pitch(y2_cur[:, :]), 1], [1, QC * CH * 16]]),
                            in_=_ap(b16, 2 * OFF_ON16, [[0, 1], [1, QC * CH * 16]]),
                        )
                    Y2 = y2_cur
                    nc.vector.tensor_scalar_max(
                        out=Y2[0:50, (i % QC) * CH * 16:(i % QC + 1) * CH * 16],
                        in0=T2[:, :], scalar1=0.0,
                    )
                    # ---- fc1 + fc2 per completed 256-sample group ----
                    if i % QC == QC - 1:
                        # All Y2 writes of this group must land before fc1
                        # reads them: sim-correct scheduling raced on HW here.
                        tc.strict_bb_all_engine_barrier()
                        NB = QC * CH  # 256
                        py2 = _pitch(Y2[:, :])
                        Y3 = y3p.tile([126, 4 * NB], F32R)
                        nc.sync.dma_start(
                            out=_ap(Y3[:, :], 125 * _pitch(Y3[:, :]),
                                    [[_pitch(Y3[:, :]), 1], [1, 4 * NB]]),
                            in_=_ap(b32, OFF_ON32, [[0, 1], [1, 4 * NB]]),
                        )
                        for c in range(4):
                            ps3 = ps3p.tile([125, NB], F32)
                            for f in range(16):
                                nc.tensor.matmul(
                                    ps3[:, :],
                                    _ap(LF1[:, :], f * 500 + c * 125,
                                        [[_pitch(LF1[:, :]), 51], [1, 125]]),
                                    _ap(Y2[:, :], f, [[py2, 51], [16, NB]]),
                                    start=(f == 0), stop=(f == 15),
                                )
                            nc.vector.tensor_scalar_max(
                                out=Y3[0:125, c * NB:(c + 1) * NB],
                                in0=ps3[:, :], scalar1=0.0,
                            )
                        ps4 = ps4p.tile([10, NB], F32)
                        for c in range(4):
                            nc.tensor.matmul(
                                ps4[:, :],
                                _ap(LF2[:, :], c * 10,
                                    [[_pitch(LF2[:, :]), 126], [1, 10]]),
                                _ap(Y3[:, :], c * NB,
                                    [[_pitch(Y3[:, :]), 126], [1, NB]]),
                                start=(c == 0), stop=(c == 3),
                            )
                        OUT = osbp.tile([10, NB], F16)
                        nc.vector.tensor_copy(out=OUT[:, :], in_=ps4[:, :])
                        nc.sync.dma_start(
                            out=_ap(out_d[:], q * NB * 10, [[1, 10], [10, NB]]),
                            in_=_ap(OUT[:, :], 0, [[_pitch(OUT[:, :]), 10], [1, NB]]),
                        )
    return nc


_NC_CACHE = None


def kernel(x, ps, conv1_w, conv1_b, conv2_w, conv2_b, fc1_w, fc1_b, fc2_w, fc2_b):
    global _NC_CACHE
    from concourse import bass_utils

    if _NC_CACHE is None:
        _NC_CACHE = build_nc()
        _NC_CACHE.finalize()
    nc = _NC_CACHE

    in_maps = build_in_maps(x, ps, conv1_w, conv1_b, conv2_w, conv2_b,
                            fc1_w, fc1_b, fc2_w, fc2_b)
    res = bass_utils.run_bass_kernel_spmd(nc, in_maps, core_ids=list(range(NCORES)))
    out = np.concatenate([r["out"] for r in res.results], axis=0)
    return out.astype(np.float32)
